# revision 36
# baseline (speedup 1.0000x reference)
"""AttnBlock (GroupNorm + single-head spatial self-attention + residual) on 8 TRN2 cores.

Sharding: data-parallel over batch — B=16 images, 2 per NeuronCore. Each core runs
an identical Bass/Tile program over its 2 images; no cross-core communication.

Per-image pipeline (all on one core, C=512 channels, HW=1024 spatial):
  1. GroupNorm(32 groups): per-channel sum/sumsq (DVE/ACT) over an fp8 x copy,
     group-combine via a tiny matmul with a 0/1 group-selector, broadcast back
     via its transpose. rstd = exp(-0.5*ln(var+eps)) on ACT — keeps every ACT
     function in the one natural_log_exp table set (no ~2.7us table swaps; the
     set choice is pinned by narrowing the table map handed to the
     insert_act_table_loads pass).
  2. q,k (C x HW, channel-partitioned) and vT (HW x C, spatial-partitioned)
     via 1x1-conv matmuls against pre-transposed weights.
  3. scores^T[j,i] = sum_c k[c,j] q[c,i]; exp (scale folded into the ACT
     activation) -> P^T; den[i] = sum_j P^T via a 32.0-vector matmul whose
     accumulating MMs are interleaved with the exp evictions.
  4. 1/den via exp(-ln(den)): ACT Ln on the 1-partition den row, ones-matmul
     broadcast of ln(den) to 128 partitions, ACT Exp(scale=-1) — this avoids
     DVE's serial ~5 cyc/elem reciprocal on a single lane.
  5. num[c,i] = sum_j vT[j,c] P^T[j,i]; proj = woT.T @ num; out = x + bo_eff +
     proj * (1/den), where bo_eff = bo + wo@bv is formed on-device once so the
     vT eviction is a plain PSUM->fp8 copy (softmax normalization and the bv
     shift both commute with the channel-wise output projection).

DMA: per-queue bandwidth is only ~55 GB/s (and the gpsimd software-DGE path is
~4x slower still), so the front-critical fp8 x16 rides sync+scalar in small
chunks, the late-needed bf16 residual copy of x rides gpsimd (image b) or the
then-idle sync/scalar (image a), and the output is stored in bf16 (host
upcasts), halving the tail store. Image a's groupnorm is emitted before image
b's stats so the in-order ACT queue can't park a's Ln behind b's squares.

The attention internals (q/k/v/scores/attn-weights) run in fp8e4m3 with
DoubleRow matmuls: each MM contracts a PAIR of 128-row k-tiles per pass,
halving tensor-engine streaming time vs bf16. Weights are pre-scaled by 32 on
the host so w*32 ~ N(0,1) sits in e4m3's normal range; the 32x factors cancel
in the softmax (exp scale /32^2) and in the numerator/denominator quotient
(the den ones-vector holds 32.0). The wo projection stays bf16 (NUM in bf16)
so the final eviction keeps its single fused scalar_tensor_tensor.

Matmul groups accumulate into 2-bank [P, 2, 512] PSUM tiles so every eviction
is one [128, 1024] pass (the ~300ns per-op engine overhead is paid half as
often). A warm-up chain of matmuls runs during the initial DMA/stats front so
the PE's HAM clock gate reaches 2.4 GHz before the first real matmul. The two
images' phases interleave as qkv(a) / scores(a) / qkv(b) / num(a) / scores(b)
/ proj(a) / num(b) / proj(b), so each image's den->ln->1/den chain hides
behind the other image's matmuls.
"""

import numpy as np
import ml_dtypes
from contextlib import ExitStack

import concourse.bass as bass
import concourse.bacc as bacc
import concourse.tile as tile
import concourse.mybir as mybir
from concourse.bass_utils import run_bass_kernel_spmd

F32 = mybir.dt.float32
AF = mybir.ActivationFunctionType
OP = mybir.AluOpType
AX = mybir.AxisListType
DRM = mybir.MatmulPerfMode.DoubleRow

B, C, H, W = 16, 512, 32, 32
HW = H * W            # 1024
G = 32                # groupnorm groups
CPG = C // G          # 16 channels per group
EPS = 1e-5
NCORES = 8
BPC = B // NCORES     # 2 images per core
P = 128               # SBUF partitions
NCT = C // P          # 4 channel tiles
GPT = P // CPG        # 8 groups per channel tile
NSB = HW // P         # 8 spatial blocks of 128
FC = 512              # matmul moving-dim chunk (one PSUM bank of fp32)
NIC = HW // FC        # 2 chunks over the spatial free dim
WS = 32.0             # fp8 weight pre-scale (w*32 ~ N(0,1))
SM_SCALE = float(C) ** -0.5 / (WS * WS)   # exp scale; q,k each carry a 32x
NWARM = 25            # warm-up matmuls covering the DMA/stats front

DT = mybir.dt.bfloat16          # residual-adjacent dtype (x, NUM, wo, out)
DT_NP = ml_dtypes.bfloat16
F8 = mybir.dt.float8e4          # attention-internals dtype (DoubleRow matmuls)
F8_NP = ml_dtypes.float8_e4m3

_CACHE: dict = {}


def _pin_act_tables():
    """Narrow the ACT table map so exp/ln/square/identity/copy resolve only to
    natural_log_exp_and_others: the insert_act_table_loads pass then emits ONE
    table load instead of thrashing between exp_and_others and natural_log
    (~2.7us per swap). Set order (and so act_func_set_id) is preserved."""
    if _CACHE.get("tables_pinned"):
        return
    orig = bacc.get_activation_tables
    pinned = {AF.Exp, AF.Ln, AF.Square, AF.Identity, AF.Copy}

    def patched(arch):
        tabs = orig(arch)
        return {
            name: (fns if name == "natural_log_exp_and_others" else (fns - pinned))
            for name, fns in tabs.items()
        }

    bacc.get_activation_tables = patched
    _CACHE["tables_pinned"] = True


def _mm(nc, out, lhsT, rhs, start, stop):
    nc.tensor.matmul(out, lhsT, rhs, start=start, stop=stop)


def _mm8(nc, out, lhsT, rhs, start, stop):
    nc.tensor.matmul(out, lhsT, rhs, start=start, stop=stop, perf_mode=DRM)


def _emit(ctx, tc, io):
    nc = tc.nc

    consts = ctx.enter_context(tc.tile_pool(name="consts", bufs=1))
    pX16 = ctx.enter_context(tc.tile_pool(name="pX16", bufs=2))
    pXB = ctx.enter_context(tc.tile_pool(name="pXB", bufs=2))
    pHN = ctx.enter_context(tc.tile_pool(name="pHN", bufs=2))
    pQ = ctx.enter_context(tc.tile_pool(name="pQ", bufs=2))
    pK = ctx.enter_context(tc.tile_pool(name="pK", bufs=2))
    pVT = ctx.enter_context(tc.tile_pool(name="pVT", bufs=2))
    pPT = ctx.enter_context(tc.tile_pool(name="pPT", bufs=2))
    pNUM = ctx.enter_context(tc.tile_pool(name="pNUM", bufs=2))
    pOUT = ctx.enter_context(tc.tile_pool(name="pOUT", bufs=2))
    pS = ctx.enter_context(tc.tile_pool(name="pS", bufs=2))
    # 2-bank matmul tiles: [P, NIC, FC] fp32, 3 in flight + one aux ring
    pmm = ctx.enter_context(tc.tile_pool(name="pmm", bufs=3, space="PSUM"))
    paux = ctx.enter_context(tc.tile_pool(name="paux", bufs=1, space="PSUM"))

    # ---- both images' fp8 x copies go out FIRST as ONE packed descriptor
    # each (4KB per-partition rows: ~185 GB/s vs ~55 GB/s for 1KB rows).
    # Image a on sync (lands ~10us); image b on scalar, issued before any ACT
    # compute exists so the descriptor generation can't stall activations.
    def emit_load16(i, q):
        X16 = pX16.tile([P, NCT, HW], F8, name=f"X16_{i}", tag="X16")
        q.dma_start(X16[:, :, :], io["x16"][i])
        return X16

    X16_0 = emit_load16(0, nc.sync)

    def load_const(name, shape, dtype=F32, q=None):
        t = consts.tile(list(shape), dtype, name=f"c_{name}")
        (q or nc.sync).dma_start(t[:], io[name][:])
        return t

    # all (P, *) vectors packed into ONE DMA — each dma_start costs ~600ns of
    # issuing-engine descriptor time that would otherwise delay weight loads
    cvec = load_const("cvec", (P, 5 * NCT + GPT))
    bq_sb = cvec[:, 0 * NCT:1 * NCT]
    bk_sb = cvec[:, 1 * NCT:2 * NCT]
    bo_sb = cvec[:, 2 * NCT:3 * NCT]
    gs_sb = cvec[:, 3 * NCT:4 * NCT]
    gb_sb = cvec[:, 4 * NCT:5 * NCT]
    gsel = cvec[:, 5 * NCT:5 * NCT + GPT]
    gselT = load_const("gselT", (GPT, P))
    bvcol = load_const("bvcol", (P, NCT), DT)

    # ---- weights (loaded once, shared by both images), one packed DMA per
    # matrix — ALL on the sync queue: the SP engine has no compute, so its
    # descriptor issuance is free, while a dma_start on the scalar engine
    # blocks the ACT pipeline ~0.7us (and a full ring blocks it for the whole
    # transfer). Ordered by need time: wq (first Q matmul) -> wk -> wv.
    # q/k/v weights are fp8 (x32) with [P, ct, c_out] layout so a
    # [:, ct:ct+2, :] slice is a DoubleRow stationary operand; wo stays bf16.
    w_sb = {}
    for wname in ("wqt", "wkt", "wvt"):
        t = consts.tile([P, NCT, C], F8, name=f"{wname}_p")
        nc.sync.dma_start(t[:, :, :], io[wname][:])
        w_sb[wname] = t
    # image b's x16 lands ~17.5us — AFTER image a's norm chain is underway, so
    # the scheduler can't hoist b's stats squares ahead of a's Ln on the
    # in-order ACT queue (data-readiness drives its priorities)
    X16_1 = emit_load16(1, nc.sync)
    wot_sb = consts.tile([P, NCT, C], DT, name="wot_p")
    nc.sync.dma_start(wot_sb[:, :, :], io["wot"][:])
    # image a's bf16 residual copy rides sync behind the weights (one 8KB-row
    # descriptor, lands ~23us, needed ~60us); it is registered to the image
    # dict in emit_load below
    XB_0 = pXB.tile([P, NCT, HW], DT, name="XB0", tag="XB")
    nc.sync.dma_start(XB_0[:, :, :], io["x"][0])

    ones_col8 = consts.tile([P, 2, 16], F8, name="ones_col8")
    nc.vector.memset(ones_col8[:], WS)   # 32.0: cancels the 32x carried by VT
    ones_row = consts.tile([1, P], DT, name="ones_row")
    nc.vector.memset(ones_row[:], 1.0)
    zb = consts.tile([P, 1], F32, name="zb")
    nc.vector.memset(zb[:], 0.0)
    epsb = consts.tile([GPT, 1], F32, name="epsb")
    nc.vector.memset(epsb[:], EPS)

    # ---- PE warm-up: a serial chain of matmuls spanning the DMA/stats front
    # keeps the HAM activity monitor busy so the clock gate opens to 2.4 GHz
    # (~3.4us in) and STAYS open until the first real matmul. Rotates through
    # the pmm ring so it costs no extra PSUM bank.
    warm8 = consts.tile([P, FC], F8, name="warm8")
    nc.vector.memset(warm8[:], 0.0)
    for w in range(NWARM):
        wp = pmm.tile([1, FC], F32, name=f"warm{w}", tag="mm")
        _mm(nc, wp[:], ones_col8[:, 0, 0:1], warm8[:], start=True, stop=True)

    # ---- per-image emission ----
    def new_img(i):
        return {"i": i}

    def emit_load(im):
        i = im["i"]
        im["X16"] = X16_0 if i == 0 else X16_1
        if i == 0:
            im["XB"] = XB_0
        else:
            # image b's residual copy rides the gpsimd software DGE (~90us of
            # slack before the proj(b) evictions need it)
            XB = pXB.tile([P, NCT, HW], DT, name=f"XB{i}", tag="XB")
            nc.gpsimd.dma_start(XB[:, :, :], io["x"][i])
            im["XB"] = XB

    def emit_stats(im):
        i = im["i"]
        X16 = im["X16"]
        stats = pS.tile([P, 2 * NCT], F32, name=f"stats{i}", tag="stats")
        scratch = pS.tile([P, HW], DT, name=f"scr{i}", tag="scratch")
        for ct in range(NCT):
            nc.vector.tensor_reduce(stats[:, ct:ct + 1], X16[:, ct, :], AX.X, OP.add)
            nc.scalar.activation(scratch[:], X16[:, ct, :], AF.Square, bias=zb[:],
                                 accum_out=stats[:, NCT + ct:NCT + ct + 1])
        im["stats"] = stats

    def emit_norm(im):
        # high_priority: the norm chain is ~12 tiny ops ping-ponging DVE<->ACT;
        # without it the scheduler interleaves the other image's 1.1us stats
        # passes between every step (+6us of pure latency on the critical path)
        i = im["i"]
        X16, stats = im["X16"], im["stats"]
        with nc.named_scope(f"norm{i}"), tc.high_priority():
            gst = paux.tile([GPT, 2 * NCT], F32, name=f"gst{i}", tag="aux")
            _mm(nc, gst[:], gsel[:], stats[:], start=True, stop=True)
            gm = pS.tile([GPT, 2 * NCT], F32, name=f"gm{i}", tag="gm")
            nc.vector.tensor_scalar_mul(gm[:], gst[:], 1.0 / (CPG * HW))
            sq = pS.tile([GPT, NCT], F32, name=f"sq{i}", tag="sq")
            nc.vector.tensor_mul(sq[:], gm[:, 0:NCT], gm[:, 0:NCT])
            var = pS.tile([GPT, NCT], F32, name=f"var{i}", tag="var")
            nc.vector.tensor_sub(var[:], gm[:, NCT:], sq[:])
            # rstd = exp(-0.5*ln(var+eps)) — Ln/Exp live in one ACT table set,
            # unlike Sqrt (whose set swap costs ~2.7us each way)
            lnv = pS.tile([GPT, NCT], F32, name=f"lnv{i}", tag="lnv")
            nc.scalar.activation(lnv[:], var[:], AF.Ln, bias=epsb[:])
            gmr = pS.tile([GPT, 2 * NCT], F32, name=f"gmr{i}", tag="gmr")
            nc.vector.tensor_copy(gmr[:, 0:NCT], gm[:, 0:NCT])
            nc.scalar.activation(gmr[:, NCT:], lnv[:], AF.Exp, bias=zb[0:GPT, :],
                                 scale=-0.5)
            pmr = paux.tile([P, 2 * NCT], F32, name=f"pmr{i}", tag="aux")
            _mm(nc, pmr[:], gselT[:], gmr[:], start=True, stop=True)
            mr = pS.tile([P, 2 * NCT], F32, name=f"mr{i}", tag="mr")
            nc.vector.tensor_copy(mr[:], pmr[:])
            # a = rstd*scale (cols NCT..), b = gn_bias - mean*a (cols 0..NCT)
            ab = pS.tile([P, 2 * NCT], F32, name=f"ab{i}", tag="ab")
            tb = pS.tile([P, NCT], F32, name=f"tb{i}", tag="tb")
            for ct in range(NCT):
                a_col = ab[:, NCT + ct:NCT + ct + 1]
                nc.vector.tensor_mul(a_col, mr[:, NCT + ct:NCT + ct + 1], gs_sb[:, ct:ct + 1])
                nc.vector.tensor_mul(tb[:, ct:ct + 1], mr[:, ct:ct + 1], a_col)
                nc.vector.tensor_sub(ab[:, ct:ct + 1], gb_sb[:, ct:ct + 1], tb[:, ct:ct + 1])
            HN = pHN.tile([P, NCT, HW], F8, name=f"HN{i}", tag="HN")
            for ct in range(NCT):
                nc.vector.tensor_scalar(HN[:, ct, :], X16[:, ct, :],
                                        ab[:, NCT + ct:NCT + ct + 1], ab[:, ct:ct + 1],
                                        OP.mult, OP.add)
            im["HN"] = HN

    def emit_boeff():
        # bo_eff = bo + wo@bv (both commute past the attention average), formed
        # once so the vT eviction needs no bias add. Emitted after norm(b) so
        # its aux-ring slots never gate the groupnorm matmuls.
        boeff = consts.tile([P, NCT], F32, name="boeff")
        for ob in range(NCT):
            ps = paux.tile([P, 1], F32, name=f"wobv{ob}", tag="aux")
            for ct in range(NCT):
                _mm(nc, ps[:], wot_sb[:, ct, ob * P:(ob + 1) * P], bvcol[:, ct:ct + 1],
                    start=(ct == 0), stop=(ct == NCT - 1))
            nc.vector.tensor_add(boeff[:, ob:ob + 1], bo_sb[:, ob:ob + 1], ps[:])
        return boeff

    def emit_qkv(im):
        i = im["i"]
        HN = im["HN"]
        with nc.named_scope(f"qkv{i}"):
            Q = pQ.tile([P, NCT, HW], F8, name=f"Q{i}", tag="Q")
            K = pK.tile([P, NCT, HW], F8, name=f"K{i}", tag="K")
            for wname, bias_sb, OT, on_act in (("wqt", bq_sb, Q, True),
                                               ("wkt", bk_sb, K, False)):
                for ob in range(NCT):
                    ps = pmm.tile([P, NIC, FC], F32, name=f"{wname}ps{i}_{ob}", tag="mm")
                    for ct in range(0, NCT, 2):
                        lhs = w_sb[wname][:, ct:ct + 2, ob * P:(ob + 1) * P]
                        for ic in range(NIC):
                            _mm8(nc, ps[:, ic, :], lhs, HN[:, ct:ct + 2, ic * FC:(ic + 1) * FC],
                                 start=(ct == 0), stop=(ct == NCT - 2))
                    # one [128,1024] eviction per ob; Q on ACT, K on DVE to
                    # balance the two engines' load
                    if on_act:
                        nc.scalar.add(OT[:, ob, :], ps[:], bias_sb[:, ob:ob + 1])
                    else:
                        nc.vector.tensor_scalar_add(OT[:, ob, :], ps[:],
                                                    bias_sb[:, ob:ob + 1])
            VT = pVT.tile([P, NSB, C], F8, name=f"VT{i}", tag="VT")
            for sb in range(0, NSB, 2):
                ps = pmm.tile([P, 2, C], F32, name=f"vtps{i}_{sb}", tag="mm")
                for k in range(2):
                    for ct in range(0, NCT, 2):
                        _mm8(nc, ps[:, k, :], HN[:, ct:ct + 2, (sb + k) * P:(sb + k + 1) * P],
                             w_sb["wvt"][:, ct:ct + 2, 0:C],
                             start=(ct == 0), stop=(ct == NCT - 2))
                nc.vector.tensor_copy(VT[:, sb:sb + 2, :], ps[:])
            im["Q"], im["K"], im["VT"] = Q, K, VT

    def emit_scores(im):
        i = im["i"]
        Q, K = im["Q"], im["K"]
        with nc.named_scope(f"scores{i}"):
            PT = pPT.tile([P, NSB, HW], F8, name=f"PT{i}", tag="PT")
            # den accumulates across jb pairs; its MMs are emitted inside the
            # jb loop so each lands right after the exp that feeds it
            den = paux.tile([1, NIC, FC], F32, name=f"den{i}", tag="aux")
            for jb in range(NSB):
                ps = pmm.tile([P, NIC, FC], F32, name=f"sps{i}_{jb}", tag="mm")
                for ct in range(0, NCT, 2):
                    lhs = K[:, ct:ct + 2, jb * P:(jb + 1) * P]
                    for ic in range(NIC):
                        _mm8(nc, ps[:, ic, :], lhs, Q[:, ct:ct + 2, ic * FC:(ic + 1) * FC],
                             start=(ct == 0), stop=(ct == NCT - 2))
                nc.scalar.activation(PT[:, jb, :], ps[:], AF.Exp, bias=zb[:],
                                     scale=SM_SCALE)
                if jb % 2 == 1:
                    for ic in range(NIC):
                        _mm8(nc, den[:, ic, :], ones_col8[:, 0:2, 0:1],
                             PT[:, jb - 1:jb + 1, ic * FC:(ic + 1) * FC],
                             start=(jb == 1), stop=(jb == NSB - 1))
            lnden = pS.tile([1, HW], DT, name=f"lnden{i}", tag="lnden")
            nc.scalar.activation(lnden[:], den[:], AF.Ln, bias=zb[0:1, :])
            im["PT"], im["lnden"] = PT, lnden

    def emit_attn_num(im):
        i = im["i"]
        VT, PT = im["VT"], im["PT"]
        with nc.named_scope(f"num{i}"):
            # num = vT.T @ P^T with the 1/den softmax normalization folded into
            # the PSUM eviction (commutes with the channel-wise wo projection);
            # 1/den arrives as exp(-lnden) with the broadcast done by a matmul
            # BETWEEN Ln and Exp so no engine touches 1 lane for long. The
            # Ln/rb/Exp chain hides behind the OTHER image's matmuls (qkv(b)
            # for image a, proj(a) for image b).
            recipb = pS.tile([P, HW], F32, name=f"recipb{i}", tag="recipb")
            rb = paux.tile([P, NIC, FC], F32, name=f"rb{i}", tag="aux")
            for ic in range(NIC):
                _mm(nc, rb[:, ic, :], ones_row[:],
                    im["lnden"][:, ic * FC:(ic + 1) * FC], start=True, stop=True)
            nc.scalar.activation(recipb[:], rb[:], AF.Exp, bias=zb[:], scale=-1.0)
            NUM = pNUM.tile([P, NCT, HW], DT, name=f"NUM{i}", tag="NUM")
            for cb in range(NCT):
                ps = pmm.tile([P, NIC, FC], F32, name=f"nps{i}_{cb}", tag="mm")
                for jt in range(0, NSB, 2):
                    lhs = VT[:, jt:jt + 2, cb * P:(cb + 1) * P]
                    for ic in range(NIC):
                        _mm8(nc, ps[:, ic, :], lhs, PT[:, jt:jt + 2, ic * FC:(ic + 1) * FC],
                             start=(jt == 0), stop=(jt == NSB - 2))
                nc.vector.tensor_mul(NUM[:, cb, :], ps[:], recipb[:])
            im["NUM"] = NUM

    def emit_attn_proj(im, boeff):
        i = im["i"]
        XB, NUM = im["XB"], im["NUM"]
        with nc.named_scope(f"proj{i}"):
            # proj + residual (+bo_eff) straight from PSUM (bf16 out, host
            # upcasts), then store each half-block on its own DMA queue
            OUTT = pOUT.tile([P, NCT, HW], DT, name=f"OUT{i}", tag="OUT")
            for ob in range(NCT):
                ps = pmm.tile([P, NIC, FC], F32, name=f"pps{i}_{ob}", tag="mm")
                for ct in range(NCT):
                    lhs = wot_sb[:, ct, ob * P:(ob + 1) * P]
                    for ic in range(NIC):
                        _mm(nc, ps[:, ic, :], lhs, NUM[:, ct, ic * FC:(ic + 1) * FC],
                            start=(ct == 0), stop=(ct == NCT - 1))
                nc.vector.scalar_tensor_tensor(OUTT[:, ob, :], ps[:],
                                               boeff[:, ob:ob + 1], XB[:, ob, :],
                                               OP.add, OP.add)
                # image a stores on sync only (ACT is busy with scores(b) exps
                # then — a scalar dma_start would stall them); image b
                # alternates sync/scalar (ACT has no work left by then).
                # Full [128,1024] bf16 blocks keep 2KB per-partition rows.
                q = nc.sync if (i == 0 or ob % 2 == 0) else nc.scalar
                q.dma_start(io["out"][i, ob * P:(ob + 1) * P, :], OUTT[:, ob, :])

    ims = [new_img(i) for i in range(BPC)]
    a, b = ims
    emit_load(a)
    emit_stats(a)
    emit_load(b)
    emit_norm(a)
    emit_qkv(a)
    emit_stats(b)       # after qkv(a): b's squares fill the ACT window
                        # between a's Q-evictions and a's exps
    emit_norm(b)
    emit_scores(a)
    boeff = emit_boeff()   # needs wot (~18us) — after scores(a) so the aux
                           # ring and PE queue aren't gated on it earlier
    emit_qkv(b)          # hides image a's den->ln->recip chain
    emit_attn_num(a)
    emit_scores(b)
    emit_attn_proj(a, boeff)   # hides image b's den->ln->recip chain
    emit_attn_num(b)
    emit_attn_proj(b, boeff)


def _build():
    if "nc" in _CACHE:
        return _CACHE["nc"]
    _pin_act_tables()
    nc = bacc.Bacc("TRN2", target_bir_lowering=False, debug=False, num_devices=NCORES)
    io = {}
    # x and x16 are host-packed to (P, NCT*HW) so each image loads as ONE
    # descriptor with >=4KB per-partition rows (~185 GB/s vs ~55 for 1KB rows)
    io["x"] = nc.dram_tensor("x", [BPC, P, NCT * HW], DT, kind="ExternalInput").ap()
    io["x16"] = nc.dram_tensor("x16", [BPC, P, NCT * HW], F8,
                               kind="ExternalInput").ap()
    for wname in ("wqt", "wkt", "wvt"):
        io[wname] = nc.dram_tensor(wname, [P, NCT, C], F8, kind="ExternalInput").ap()
    io["wot"] = nc.dram_tensor("wot", [P, NCT, C], DT, kind="ExternalInput").ap()
    io["cvec"] = nc.dram_tensor("cvec", [P, 5 * NCT + GPT], F32,
                                kind="ExternalInput").ap()
    io["bvcol"] = nc.dram_tensor("bvcol", [P, NCT], DT, kind="ExternalInput").ap()
    io["gselT"] = nc.dram_tensor("gselT", [GPT, P], F32, kind="ExternalInput").ap()
    io["out"] = nc.dram_tensor("out", [BPC, C, HW], DT, kind="ExternalOutput").ap()

    with tile.TileContext(nc) as tc:
        with ExitStack() as ctx:
            _emit(ctx, tc, io)
    nc.compile()
    _CACHE["nc"] = nc
    return nc


def _col_layout(v):
    # (C,) -> (P, NCT): column ct holds channels [ct*128, (ct+1)*128)
    return np.ascontiguousarray(np.asarray(v, np.float32).reshape(NCT, P).T)


def _run(inputs, trace=False, **run_kwargs):
    x = np.ascontiguousarray(np.asarray(inputs["x"], np.float32).reshape(B, C, HW))
    def _wpack(w, scale, npdt):
        # wT (c_in, c_out) -> (P, NCT, C): W[p, ct, j] = wT[ct*128+p, j] * scale
        wt = (np.asarray(w, np.float32).T * scale).astype(npdt)
        return np.ascontiguousarray(wt.reshape(NCT, P, C).transpose(1, 0, 2))

    wdt = {n: _wpack(inputs[s], WS, F8_NP)
           for n, s in (("wqt", "wq"), ("wkt", "wk"), ("wvt", "wv"))}
    wdt["wot"] = _wpack(inputs["wo"], 1.0, DT_NP)
    pidx = np.arange(P)
    gsel = (pidx[:, None] // CPG == np.arange(GPT)[None, :]).astype(np.float32)
    # bq/bk carry the 32x weight scale so Q=32q, K=32k on-device; bv is folded
    # into bo_eff on-device (bo + wo@bv) so vT needs no bias at all
    cvec = np.concatenate([_col_layout(np.asarray(inputs["bq"]) * WS),
                           _col_layout(np.asarray(inputs["bk"]) * WS),
                           _col_layout(inputs["bo"]), _col_layout(inputs["gn_scale"]),
                           _col_layout(inputs["gn_bias"]), gsel], axis=1)
    common = {
        **wdt,
        "cvec": np.ascontiguousarray(cvec),
        "bvcol": np.ascontiguousarray(_col_layout(inputs["bv"]).astype(DT_NP)),
        "gselT": np.ascontiguousarray(gsel.T),
    }
    # pack (B, C, HW) -> (B, P, NCT*HW): row p holds channels p, 128+p, ...
    xp = x.reshape(B, NCT, P, HW).transpose(0, 2, 1, 3).reshape(B, P, NCT * HW)
    xb = xp.astype(DT_NP)
    x16 = xp.astype(F8_NP)
    in_maps = [{"x": np.ascontiguousarray(xb[m * BPC:(m + 1) * BPC]),
                "x16": np.ascontiguousarray(x16[m * BPC:(m + 1) * BPC]), **common}
               for m in range(NCORES)]
    nc = _build()
    res = run_bass_kernel_spmd(nc, in_maps, core_ids=list(range(NCORES)),
                               trace=trace, **run_kwargs)
    out = np.concatenate([r["out"] for r in res.results], axis=0)
    return out.reshape(B, C, H, W).astype(np.float32), res


def kernel(**inputs):
    out, _ = _run(inputs)
    return out


# revision 37
# speedup vs baseline: 1.0443x; 1.0443x over previous
"""AttnBlock (GroupNorm + single-head spatial self-attention + residual) on 8 TRN2 cores.

Sharding: data-parallel over batch — B=16 images, 2 per NeuronCore. Each core runs
an identical Bass/Tile program over its 2 images; no cross-core communication.

Per-image pipeline (all on one core, C=512 channels, HW=1024 spatial):
  1. GroupNorm(32 groups): per-channel sum/sumsq (DVE/ACT) over an fp8 x copy,
     group-combine via a tiny matmul with a 0/1 group-selector, broadcast back
     via its transpose. rstd = exp(-0.5*ln(var+eps)) on ACT — keeps every ACT
     function in the one natural_log_exp table set (no ~2.7us table swaps; the
     set choice is pinned by narrowing the table map handed to the
     insert_act_table_loads pass).
  2. q,k (C x HW, channel-partitioned) and vT (HW x C, spatial-partitioned)
     via 1x1-conv matmuls against pre-transposed weights.
  3. scores^T[j,i] = sum_c k[c,j] q[c,i]; exp (scale folded into the ACT
     activation) -> P^T; den[i] = sum_j P^T via a 32.0-vector matmul whose
     accumulating MMs are interleaved with the exp evictions.
  4. 1/den via exp(-ln(den)): ACT Ln on the 1-partition den row, ones-matmul
     broadcast of ln(den) to 128 partitions, ACT Exp(scale=-1) — this avoids
     DVE's serial ~5 cyc/elem reciprocal on a single lane.
  5. num[c,i] = sum_j vT[j,c] P^T[j,i]; proj = woT.T @ num; out = x + bo_eff +
     proj * (1/den), where bo_eff = bo + wo@bv is formed on-device once so the
     vT eviction is a plain PSUM->fp8 copy (softmax normalization and the bv
     shift both commute with the channel-wise output projection).

DMA: per-queue bandwidth is only ~55 GB/s (and the gpsimd software-DGE path is
~4x slower still), so the front-critical fp8 x16 rides sync+scalar in small
chunks, the late-needed bf16 residual copy of x rides gpsimd (image b) or the
then-idle sync/scalar (image a), and the output is stored in bf16 (host
upcasts), halving the tail store. Image a's groupnorm is emitted before image
b's stats so the in-order ACT queue can't park a's Ln behind b's squares.

The attention internals (q/k/v/scores/attn-weights) run in fp8e4m3 with
DoubleRow matmuls: each MM contracts a PAIR of 128-row k-tiles per pass,
halving tensor-engine streaming time vs bf16. Weights are pre-scaled by 32 on
the host so w*32 ~ N(0,1) sits in e4m3's normal range; the 32x factors cancel
in the softmax (exp scale /32^2) and in the numerator/denominator quotient
(the den ones-vector holds 32.0). The wo projection stays bf16 (NUM in bf16)
so the final eviction keeps its single fused scalar_tensor_tensor.

Matmul groups accumulate into 2-bank [P, 2, 512] PSUM tiles so every eviction
is one [128, 1024] pass (the ~300ns per-op engine overhead is paid half as
often). A warm-up chain of matmuls runs during the initial DMA/stats front so
the PE's HAM clock gate reaches 2.4 GHz before the first real matmul. The two
images' phases interleave as qkv(a) / scores(a) / qkv(b) / num(a) / scores(b)
/ proj(a) / num(b) / proj(b), so each image's den->ln->1/den chain hides
behind the other image's matmuls.
"""

import numpy as np
import ml_dtypes
from contextlib import ExitStack

import concourse.bass as bass
import concourse.bacc as bacc
import concourse.tile as tile
import concourse.mybir as mybir
from concourse.bass_utils import run_bass_kernel_spmd

F32 = mybir.dt.float32
AF = mybir.ActivationFunctionType
OP = mybir.AluOpType
AX = mybir.AxisListType
DRM = mybir.MatmulPerfMode.DoubleRow

B, C, H, W = 16, 512, 32, 32
HW = H * W            # 1024
G = 32                # groupnorm groups
CPG = C // G          # 16 channels per group
EPS = 1e-5
NCORES = 8
BPC = B // NCORES     # 2 images per core
P = 128               # SBUF partitions
NCT = C // P          # 4 channel tiles
GPT = P // CPG        # 8 groups per channel tile
NSB = HW // P         # 8 spatial blocks of 128
FC = 512              # matmul moving-dim chunk (one PSUM bank of fp32)
NIC = HW // FC        # 2 chunks over the spatial free dim
WS = 32.0             # fp8 weight pre-scale (w*32 ~ N(0,1))
SM_SCALE = float(C) ** -0.5 / (WS * WS)   # exp scale; q,k each carry a 32x
NWARM = 25            # warm-up matmuls covering the DMA/stats front

DT = mybir.dt.bfloat16          # residual-adjacent dtype (x, NUM, wo, out)
DT_NP = ml_dtypes.bfloat16
F8 = mybir.dt.float8e4          # attention-internals dtype (DoubleRow matmuls)
F8_NP = ml_dtypes.float8_e4m3

_CACHE: dict = {}


def _pin_act_tables():
    """Narrow the ACT table map so exp/ln/square/identity/copy resolve only to
    natural_log_exp_and_others: the insert_act_table_loads pass then emits ONE
    table load instead of thrashing between exp_and_others and natural_log
    (~2.7us per swap). Set order (and so act_func_set_id) is preserved."""
    if _CACHE.get("tables_pinned"):
        return
    orig = bacc.get_activation_tables
    pinned = {AF.Exp, AF.Ln, AF.Square, AF.Identity, AF.Copy}

    def patched(arch):
        tabs = orig(arch)
        return {
            name: (fns if name == "natural_log_exp_and_others" else (fns - pinned))
            for name, fns in tabs.items()
        }

    bacc.get_activation_tables = patched
    _CACHE["tables_pinned"] = True


def _mm(nc, out, lhsT, rhs, start, stop):
    nc.tensor.matmul(out, lhsT, rhs, start=start, stop=stop)


def _mm8(nc, out, lhsT, rhs, start, stop):
    nc.tensor.matmul(out, lhsT, rhs, start=start, stop=stop, perf_mode=DRM)


def _emit(ctx, tc, io):
    nc = tc.nc

    consts = ctx.enter_context(tc.tile_pool(name="consts", bufs=1))
    pX16 = ctx.enter_context(tc.tile_pool(name="pX16", bufs=2))
    pXB = ctx.enter_context(tc.tile_pool(name="pXB", bufs=2))
    pHN = ctx.enter_context(tc.tile_pool(name="pHN", bufs=2))
    pQ = ctx.enter_context(tc.tile_pool(name="pQ", bufs=2))
    pK = ctx.enter_context(tc.tile_pool(name="pK", bufs=2))
    pVT = ctx.enter_context(tc.tile_pool(name="pVT", bufs=2))
    pPT = ctx.enter_context(tc.tile_pool(name="pPT", bufs=2))
    pNUM = ctx.enter_context(tc.tile_pool(name="pNUM", bufs=2))
    pOUT = ctx.enter_context(tc.tile_pool(name="pOUT", bufs=2))
    pS = ctx.enter_context(tc.tile_pool(name="pS", bufs=2))
    # 2-bank matmul tiles: [P, NIC, FC] fp32, 3 in flight + one aux ring
    pmm = ctx.enter_context(tc.tile_pool(name="pmm", bufs=3, space="PSUM"))
    paux = ctx.enter_context(tc.tile_pool(name="paux", bufs=1, space="PSUM"))

    # ---- both images' fp8 x copies go out FIRST as ONE packed descriptor
    # each (4KB per-partition rows: ~185 GB/s vs ~55 GB/s for 1KB rows).
    # Image a on sync (lands ~10us); image b on scalar, issued before any ACT
    # compute exists so the descriptor generation can't stall activations.
    def emit_load16(i, q):
        X16 = pX16.tile([P, NCT, HW], F8, name=f"X16_{i}", tag="X16")
        q.dma_start(X16[:, :, :], io["x16"][i])
        return X16

    X16_0 = emit_load16(0, nc.sync)

    def load_const(name, shape, dtype=F32, q=None):
        t = consts.tile(list(shape), dtype, name=f"c_{name}")
        (q or nc.sync).dma_start(t[:], io[name][:])
        return t

    # all (P, *) vectors packed into ONE DMA — each dma_start costs ~600ns of
    # issuing-engine descriptor time that would otherwise delay weight loads
    cvec = load_const("cvec", (P, 5 * NCT + GPT))
    bq_sb = cvec[:, 0 * NCT:1 * NCT]
    bk_sb = cvec[:, 1 * NCT:2 * NCT]
    bo_sb = cvec[:, 2 * NCT:3 * NCT]
    gs_sb = cvec[:, 3 * NCT:4 * NCT]
    gb_sb = cvec[:, 4 * NCT:5 * NCT]
    gsel = cvec[:, 5 * NCT:5 * NCT + GPT]
    gselT = load_const("gselT", (GPT, P))
    bvcol = load_const("bvcol", (P, NCT), DT)

    # ---- weights (loaded once, shared by both images), one packed DMA per
    # matrix — ALL on the sync queue: the SP engine has no compute, so its
    # descriptor issuance is free, while a dma_start on the scalar engine
    # blocks the ACT pipeline ~0.7us (and a full ring blocks it for the whole
    # transfer). Ordered by need time: wq (first Q matmul) -> wk -> wv.
    # q/k/v weights are fp8 (x32) with [P, ct, c_out] layout so a
    # [:, ct:ct+2, :] slice is a DoubleRow stationary operand; wo stays bf16.
    w_sb = {}
    for wname in ("wqt", "wkt", "wvt"):
        t = consts.tile([P, NCT, C], F8, name=f"{wname}_p")
        nc.sync.dma_start(t[:, :, :], io[wname][:])
        w_sb[wname] = t
    # image b's x16 lands ~17.5us — AFTER image a's norm chain is underway, so
    # the scheduler can't hoist b's stats squares ahead of a's Ln on the
    # in-order ACT queue (data-readiness drives its priorities)
    X16_1 = emit_load16(1, nc.sync)
    wot_sb = consts.tile([P, NCT, C], DT, name="wot_p")
    nc.sync.dma_start(wot_sb[:, :, :], io["wot"][:])
    # image a's bf16 residual copy rides sync behind the weights (one 8KB-row
    # descriptor, lands ~23us, needed ~60us); it is registered to the image
    # dict in emit_load below
    XB_0 = pXB.tile([P, NCT, HW], DT, name="XB0", tag="XB")
    nc.sync.dma_start(XB_0[:, :, :], io["x"][0])

    ones_col8 = consts.tile([P, 2, 16], F8, name="ones_col8")
    nc.vector.memset(ones_col8[:], WS)   # 32.0: cancels the 32x carried by VT
    ones_row = consts.tile([1, P], DT, name="ones_row")
    nc.vector.memset(ones_row[:], 1.0)
    zb = consts.tile([P, 1], F32, name="zb")
    nc.vector.memset(zb[:], 0.0)
    epsb = consts.tile([GPT, 1], F32, name="epsb")
    nc.vector.memset(epsb[:], EPS)

    # ---- PE warm-up: a serial chain of matmuls spanning the DMA/stats front
    # keeps the HAM activity monitor busy so the clock gate opens to 2.4 GHz
    # (~3.4us in) and STAYS open until the first real matmul. Rotates through
    # the pmm ring so it costs no extra PSUM bank.
    warm8 = consts.tile([P, FC], F8, name="warm8")
    nc.vector.memset(warm8[:], 0.0)
    for w in range(NWARM):
        wp = pmm.tile([1, FC], F32, name=f"warm{w}", tag="mm")
        _mm(nc, wp[:], ones_col8[:, 0, 0:1], warm8[:], start=True, stop=True)

    # ---- per-image emission ----
    def new_img(i):
        return {"i": i}

    def emit_load(im):
        i = im["i"]
        im["X16"] = X16_0 if i == 0 else X16_1
        if i == 0:
            im["XB"] = XB_0
        else:
            # image b's residual copy rides the gpsimd software DGE (~90us of
            # slack before the proj(b) evictions need it)
            XB = pXB.tile([P, NCT, HW], DT, name=f"XB{i}", tag="XB")
            nc.gpsimd.dma_start(XB[:, :, :], io["x"][i])
            im["XB"] = XB

    def emit_stats(im):
        i = im["i"]
        X16 = im["X16"]
        stats = pS.tile([P, 2 * NCT], F32, name=f"stats{i}", tag="stats")
        scratch = pS.tile([P, HW], DT, name=f"scr{i}", tag="scratch")
        for ct in range(NCT):
            nc.vector.tensor_reduce(stats[:, ct:ct + 1], X16[:, ct, :], AX.X, OP.add)
            nc.scalar.activation(scratch[:], X16[:, ct, :], AF.Square, bias=zb[:],
                                 accum_out=stats[:, NCT + ct:NCT + ct + 1])
        im["stats"] = stats

    def emit_norm(im):
        # high_priority: the norm chain is ~12 tiny ops ping-ponging DVE<->ACT;
        # without it the scheduler interleaves the other image's 1.1us stats
        # passes between every step (+6us of pure latency on the critical path)
        i = im["i"]
        X16, stats = im["X16"], im["stats"]
        with nc.named_scope(f"norm{i}"), tc.high_priority():
            gst = paux.tile([GPT, 2 * NCT], F32, name=f"gst{i}", tag="aux")
            _mm(nc, gst[:], gsel[:], stats[:], start=True, stop=True)
            gm = pS.tile([GPT, 2 * NCT], F32, name=f"gm{i}", tag="gm")
            nc.vector.tensor_scalar_mul(gm[:], gst[:], 1.0 / (CPG * HW))
            sq = pS.tile([GPT, NCT], F32, name=f"sq{i}", tag="sq")
            nc.vector.tensor_mul(sq[:], gm[:, 0:NCT], gm[:, 0:NCT])
            var = pS.tile([GPT, NCT], F32, name=f"var{i}", tag="var")
            nc.vector.tensor_sub(var[:], gm[:, NCT:], sq[:])
            # rstd = exp(-0.5*ln(var+eps)) — Ln/Exp live in one ACT table set,
            # unlike Sqrt (whose set swap costs ~2.7us each way)
            lnv = pS.tile([GPT, NCT], F32, name=f"lnv{i}", tag="lnv")
            nc.scalar.activation(lnv[:], var[:], AF.Ln, bias=epsb[:])
            gmr = pS.tile([GPT, 2 * NCT], F32, name=f"gmr{i}", tag="gmr")
            nc.vector.tensor_copy(gmr[:, 0:NCT], gm[:, 0:NCT])
            nc.scalar.activation(gmr[:, NCT:], lnv[:], AF.Exp, bias=zb[0:GPT, :],
                                 scale=-0.5)
            pmr = paux.tile([P, 2 * NCT], F32, name=f"pmr{i}", tag="aux")
            _mm(nc, pmr[:], gselT[:], gmr[:], start=True, stop=True)
            mr = pS.tile([P, 2 * NCT], F32, name=f"mr{i}", tag="mr")
            nc.vector.tensor_copy(mr[:], pmr[:])
            # a = rstd*scale (cols NCT..), b = gn_bias - mean*a (cols 0..NCT)
            ab = pS.tile([P, 2 * NCT], F32, name=f"ab{i}", tag="ab")
            tb = pS.tile([P, NCT], F32, name=f"tb{i}", tag="tb")
            for ct in range(NCT):
                a_col = ab[:, NCT + ct:NCT + ct + 1]
                nc.vector.tensor_mul(a_col, mr[:, NCT + ct:NCT + ct + 1], gs_sb[:, ct:ct + 1])
                nc.vector.tensor_mul(tb[:, ct:ct + 1], mr[:, ct:ct + 1], a_col)
                nc.vector.tensor_sub(ab[:, ct:ct + 1], gb_sb[:, ct:ct + 1], tb[:, ct:ct + 1])
            HN = pHN.tile([P, NCT, HW], F8, name=f"HN{i}", tag="HN")
            for ct in range(NCT):
                nc.vector.tensor_scalar(HN[:, ct, :], X16[:, ct, :],
                                        ab[:, NCT + ct:NCT + ct + 1], ab[:, ct:ct + 1],
                                        OP.mult, OP.add)
            im["HN"] = HN

    def emit_boeff():
        # bo_eff = bo + wo@bv (both commute past the attention average), formed
        # once so the vT eviction needs no bias add. Emitted after norm(b) so
        # its aux-ring slots never gate the groupnorm matmuls.
        boeff = consts.tile([P, NCT], F32, name="boeff")
        for ob in range(NCT):
            ps = paux.tile([P, 1], F32, name=f"wobv{ob}", tag="aux")
            for ct in range(NCT):
                _mm(nc, ps[:], wot_sb[:, ct, ob * P:(ob + 1) * P], bvcol[:, ct:ct + 1],
                    start=(ct == 0), stop=(ct == NCT - 1))
            nc.vector.tensor_add(boeff[:, ob:ob + 1], bo_sb[:, ob:ob + 1], ps[:])
        return boeff

    def emit_qkv(im):
        i = im["i"]
        HN = im["HN"]
        with nc.named_scope(f"qkv{i}"):
            Q = pQ.tile([P, NCT, HW], F8, name=f"Q{i}", tag="Q")
            K = pK.tile([P, NCT, HW], F8, name=f"K{i}", tag="K")
            for wname, bias_sb, OT, on_act in (("wqt", bq_sb, Q, True),
                                               ("wkt", bk_sb, K, False)):
                for ob in range(NCT):
                    ps = pmm.tile([P, NIC, FC], F32, name=f"{wname}ps{i}_{ob}", tag="mm")
                    for ct in range(0, NCT, 2):
                        lhs = w_sb[wname][:, ct:ct + 2, ob * P:(ob + 1) * P]
                        for ic in range(NIC):
                            _mm8(nc, ps[:, ic, :], lhs, HN[:, ct:ct + 2, ic * FC:(ic + 1) * FC],
                                 start=(ct == 0), stop=(ct == NCT - 2))
                    # one [128,1024] eviction per ob; Q on ACT, K on DVE to
                    # balance the two engines' load
                    if on_act:
                        nc.scalar.add(OT[:, ob, :], ps[:], bias_sb[:, ob:ob + 1])
                    else:
                        nc.vector.tensor_scalar_add(OT[:, ob, :], ps[:],
                                                    bias_sb[:, ob:ob + 1])
            VT = pVT.tile([P, NSB, C], F8, name=f"VT{i}", tag="VT")
            for sb in range(0, NSB, 2):
                ps = pmm.tile([P, 2, C], F32, name=f"vtps{i}_{sb}", tag="mm")
                for k in range(2):
                    for ct in range(0, NCT, 2):
                        _mm8(nc, ps[:, k, :], HN[:, ct:ct + 2, (sb + k) * P:(sb + k + 1) * P],
                             w_sb["wvt"][:, ct:ct + 2, 0:C],
                             start=(ct == 0), stop=(ct == NCT - 2))
                nc.vector.tensor_copy(VT[:, sb:sb + 2, :], ps[:])
            im["Q"], im["K"], im["VT"] = Q, K, VT

    def emit_scores(im):
        i = im["i"]
        Q, K = im["Q"], im["K"]
        with nc.named_scope(f"scores{i}"):
            PT = pPT.tile([P, NSB, HW], F8, name=f"PT{i}", tag="PT")
            # den accumulates across jb pairs; its MMs are emitted inside the
            # jb loop so each lands right after the exp that feeds it
            den = paux.tile([1, NIC, FC], F32, name=f"den{i}", tag="aux")
            for jb in range(NSB):
                ps = pmm.tile([P, NIC, FC], F32, name=f"sps{i}_{jb}", tag="mm")
                for ct in range(0, NCT, 2):
                    lhs = K[:, ct:ct + 2, jb * P:(jb + 1) * P]
                    for ic in range(NIC):
                        _mm8(nc, ps[:, ic, :], lhs, Q[:, ct:ct + 2, ic * FC:(ic + 1) * FC],
                             start=(ct == 0), stop=(ct == NCT - 2))
                nc.scalar.activation(PT[:, jb, :], ps[:], AF.Exp, bias=zb[:],
                                     scale=SM_SCALE)
                if jb % 2 == 1:
                    for ic in range(NIC):
                        _mm8(nc, den[:, ic, :], ones_col8[:, 0:2, 0:1],
                             PT[:, jb - 1:jb + 1, ic * FC:(ic + 1) * FC],
                             start=(jb == 1), stop=(jb == NSB - 1))
            lnden = pS.tile([1, HW], DT, name=f"lnden{i}", tag="lnden")
            nc.scalar.activation(lnden[:], den[:], AF.Ln, bias=zb[0:1, :])
            im["PT"], im["lnden"] = PT, lnden

    def emit_attn_num(im):
        i = im["i"]
        VT, PT = im["VT"], im["PT"]
        with nc.named_scope(f"num{i}"):
            # num = vT.T @ P^T with the 1/den softmax normalization folded into
            # the PSUM eviction (commutes with the channel-wise wo projection);
            # 1/den arrives as exp(-lnden) with the broadcast done by a matmul
            # BETWEEN Ln and Exp so no engine touches 1 lane for long. The
            # Ln/rb/Exp chain hides behind the OTHER image's matmuls (qkv(b)
            # for image a, proj(a) for image b).
            recipb = pS.tile([P, HW], F32, name=f"recipb{i}", tag="recipb")
            rb = paux.tile([P, NIC, FC], F32, name=f"rb{i}", tag="aux")
            for ic in range(NIC):
                _mm(nc, rb[:, ic, :], ones_row[:],
                    im["lnden"][:, ic * FC:(ic + 1) * FC], start=True, stop=True)
            nc.scalar.activation(recipb[:], rb[:], AF.Exp, bias=zb[:], scale=-1.0)
            NUM = pNUM.tile([P, NCT, HW], DT, name=f"NUM{i}", tag="NUM")
            for cb in range(NCT):
                ps = pmm.tile([P, NIC, FC], F32, name=f"nps{i}_{cb}", tag="mm")
                for jt in range(0, NSB, 2):
                    lhs = VT[:, jt:jt + 2, cb * P:(cb + 1) * P]
                    for ic in range(NIC):
                        _mm8(nc, ps[:, ic, :], lhs, PT[:, jt:jt + 2, ic * FC:(ic + 1) * FC],
                             start=(jt == 0), stop=(jt == NSB - 2))
                nc.vector.tensor_mul(NUM[:, cb, :], ps[:], recipb[:])
            im["NUM"] = NUM

    def emit_attn_proj(im, boeff):
        i = im["i"]
        XB, NUM = im["XB"], im["NUM"]
        with nc.named_scope(f"proj{i}"):
            # proj + residual (+bo_eff) straight from PSUM (bf16 out, host
            # upcasts), then store each half-block on its own DMA queue
            OUTT = pOUT.tile([P, NCT, HW], DT, name=f"OUT{i}", tag="OUT")
            for ob in range(NCT):
                ps = pmm.tile([P, NIC, FC], F32, name=f"pps{i}_{ob}", tag="mm")
                for ct in range(NCT):
                    lhs = wot_sb[:, ct, ob * P:(ob + 1) * P]
                    for ic in range(NIC):
                        _mm(nc, ps[:, ic, :], lhs, NUM[:, ct, ic * FC:(ic + 1) * FC],
                            start=(ct == 0), stop=(ct == NCT - 1))
                nc.vector.scalar_tensor_tensor(OUTT[:, ob, :], ps[:],
                                               boeff[:, ob:ob + 1], XB[:, ob, :],
                                               OP.add, OP.add)
                # image a stores on sync only (ACT is busy with scores(b) exps
                # then — a scalar dma_start would stall them); image b
                # alternates sync/scalar (ACT has no work left by then).
                # Full [128,1024] bf16 blocks keep 2KB per-partition rows.
                q = nc.sync if (i == 0 or ob % 2 == 0) else nc.scalar
                q.dma_start(io["out"][i, ob * P:(ob + 1) * P, :], OUTT[:, ob, :])

    ims = [new_img(i) for i in range(BPC)]
    a, b = ims
    emit_load(a)
    emit_stats(a)
    emit_load(b)
    emit_norm(a)
    emit_qkv(a)
    with tc.tile_wait_until(0.018):
        # schedule-time floor of ~18us: keeps b's 1.1us stats passes OUT of
        # the ACT/DVE streams while image a's latency-critical norm chain
        # ping-pongs between those engines (the scheduler's DMA cost model
        # is ~10x optimistic, so without this b's stats look ready at ~2us)
        emit_stats(b)
    emit_norm(b)
    emit_scores(a)
    boeff = emit_boeff()   # needs wot (~18us) — after scores(a) so the aux
                           # ring and PE queue aren't gated on it earlier
    emit_qkv(b)          # hides image a's den->ln->recip chain
    emit_attn_num(a)
    emit_scores(b)
    emit_attn_proj(a, boeff)   # hides image b's den->ln->recip chain
    emit_attn_num(b)
    emit_attn_proj(b, boeff)


def _build():
    if "nc" in _CACHE:
        return _CACHE["nc"]
    _pin_act_tables()
    nc = bacc.Bacc("TRN2", target_bir_lowering=False, debug=False, num_devices=NCORES)
    io = {}
    # x and x16 are host-packed to (P, NCT*HW) so each image loads as ONE
    # descriptor with >=4KB per-partition rows (~185 GB/s vs ~55 for 1KB rows)
    io["x"] = nc.dram_tensor("x", [BPC, P, NCT * HW], DT, kind="ExternalInput").ap()
    io["x16"] = nc.dram_tensor("x16", [BPC, P, NCT * HW], F8,
                               kind="ExternalInput").ap()
    for wname in ("wqt", "wkt", "wvt"):
        io[wname] = nc.dram_tensor(wname, [P, NCT, C], F8, kind="ExternalInput").ap()
    io["wot"] = nc.dram_tensor("wot", [P, NCT, C], DT, kind="ExternalInput").ap()
    io["cvec"] = nc.dram_tensor("cvec", [P, 5 * NCT + GPT], F32,
                                kind="ExternalInput").ap()
    io["bvcol"] = nc.dram_tensor("bvcol", [P, NCT], DT, kind="ExternalInput").ap()
    io["gselT"] = nc.dram_tensor("gselT", [GPT, P], F32, kind="ExternalInput").ap()
    io["out"] = nc.dram_tensor("out", [BPC, C, HW], DT, kind="ExternalOutput").ap()

    with tile.TileContext(nc) as tc:
        with ExitStack() as ctx:
            _emit(ctx, tc, io)
    nc.compile()
    _CACHE["nc"] = nc
    return nc


def _col_layout(v):
    # (C,) -> (P, NCT): column ct holds channels [ct*128, (ct+1)*128)
    return np.ascontiguousarray(np.asarray(v, np.float32).reshape(NCT, P).T)


def _run(inputs, trace=False, **run_kwargs):
    x = np.ascontiguousarray(np.asarray(inputs["x"], np.float32).reshape(B, C, HW))
    def _wpack(w, scale, npdt):
        # wT (c_in, c_out) -> (P, NCT, C): W[p, ct, j] = wT[ct*128+p, j] * scale
        wt = (np.asarray(w, np.float32).T * scale).astype(npdt)
        return np.ascontiguousarray(wt.reshape(NCT, P, C).transpose(1, 0, 2))

    wdt = {n: _wpack(inputs[s], WS, F8_NP)
           for n, s in (("wqt", "wq"), ("wkt", "wk"), ("wvt", "wv"))}
    wdt["wot"] = _wpack(inputs["wo"], 1.0, DT_NP)
    pidx = np.arange(P)
    gsel = (pidx[:, None] // CPG == np.arange(GPT)[None, :]).astype(np.float32)
    # bq/bk carry the 32x weight scale so Q=32q, K=32k on-device; bv is folded
    # into bo_eff on-device (bo + wo@bv) so vT needs no bias at all
    cvec = np.concatenate([_col_layout(np.asarray(inputs["bq"]) * WS),
                           _col_layout(np.asarray(inputs["bk"]) * WS),
                           _col_layout(inputs["bo"]), _col_layout(inputs["gn_scale"]),
                           _col_layout(inputs["gn_bias"]), gsel], axis=1)
    common = {
        **wdt,
        "cvec": np.ascontiguousarray(cvec),
        "bvcol": np.ascontiguousarray(_col_layout(inputs["bv"]).astype(DT_NP)),
        "gselT": np.ascontiguousarray(gsel.T),
    }
    # pack (B, C, HW) -> (B, P, NCT*HW): row p holds channels p, 128+p, ...
    xp = x.reshape(B, NCT, P, HW).transpose(0, 2, 1, 3).reshape(B, P, NCT * HW)
    xb = xp.astype(DT_NP)
    x16 = xp.astype(F8_NP)
    in_maps = [{"x": np.ascontiguousarray(xb[m * BPC:(m + 1) * BPC]),
                "x16": np.ascontiguousarray(x16[m * BPC:(m + 1) * BPC]), **common}
               for m in range(NCORES)]
    nc = _build()
    res = run_bass_kernel_spmd(nc, in_maps, core_ids=list(range(NCORES)),
                               trace=trace, **run_kwargs)
    out = np.concatenate([r["out"] for r in res.results], axis=0)
    return out.reshape(B, C, H, W).astype(np.float32), res


def kernel(**inputs):
    out, _ = _run(inputs)
    return out


# revision 48
# speedup vs baseline: 1.0817x; 1.0358x over previous
"""AttnBlock (GroupNorm + single-head spatial self-attention + residual) on 8 TRN2 cores.

Sharding: data-parallel over batch — B=16 images, 2 per NeuronCore. Each core runs
an identical Bass/Tile program over its 2 images; no cross-core communication.

Per-image pipeline (all on one core, C=512 channels, HW=1024 spatial):
  1. GroupNorm(32 groups): per-channel sum/sumsq (DVE/ACT) over an fp8 x copy,
     group-combine via a tiny matmul with a 0/1 group-selector, broadcast back
     via its transpose. rstd = exp(-0.5*ln(var+eps)) on ACT — keeps every ACT
     function in the one natural_log_exp table set (no ~2.7us table swaps; the
     set choice is pinned by narrowing the table map handed to the
     insert_act_table_loads pass).
  2. q,k (C x HW, channel-partitioned) and vT (HW x C, spatial-partitioned)
     via 1x1-conv matmuls against pre-transposed weights.
  3. scores^T[j,i] = sum_c k[c,j] q[c,i]; exp (scale folded into the ACT
     activation) -> P^T; den[i] = sum_j P^T via a 32.0-vector matmul whose
     accumulating MMs are interleaved with the exp evictions.
  4. 1/den via exp(-ln(den)): ACT Ln on the 1-partition den row, ones-matmul
     broadcast of ln(den) to 128 partitions, ACT Exp(scale=-1) — this avoids
     DVE's serial ~5 cyc/elem reciprocal on a single lane.
  5. num[c,i] = sum_j vT[j,c] P^T[j,i]; proj = woT.T @ num; out = x + bo_eff +
     proj * (1/den), where bo_eff = bo + wo@bv is formed on-device once so the
     vT eviction is a plain PSUM->fp8 copy (softmax normalization and the bv
     shift both commute with the channel-wise output projection).

DMA: per-queue bandwidth is only ~55 GB/s (and the gpsimd software-DGE path is
~4x slower still), so the front-critical fp8 x16 rides sync+scalar in small
chunks, the late-needed bf16 residual copy of x rides gpsimd (image b) or the
then-idle sync/scalar (image a), and the output is stored in bf16 (host
upcasts), halving the tail store. Image a's groupnorm is emitted before image
b's stats so the in-order ACT queue can't park a's Ln behind b's squares.

The attention internals (q/k/v/scores/attn-weights) run in fp8e4m3 with
DoubleRow matmuls: each MM contracts a PAIR of 128-row k-tiles per pass,
halving tensor-engine streaming time vs bf16. Weights are pre-scaled by 32 on
the host so w*32 ~ N(0,1) sits in e4m3's normal range; the 32x factors cancel
in the softmax (exp scale /32^2) and in the numerator/denominator quotient
(the den ones-vector holds 32.0). The wo projection stays bf16 (NUM in bf16)
so the final eviction keeps its single fused scalar_tensor_tensor.

Matmul groups accumulate into 2-bank [P, 2, 512] PSUM tiles so every eviction
is one [128, 1024] pass (the ~300ns per-op engine overhead is paid half as
often). A warm-up chain of matmuls runs during the initial DMA/stats front so
the PE's HAM clock gate reaches 2.4 GHz before the first real matmul. The two
images' phases interleave as qkv(a) / scores(a) / qkv(b) / num(a) / scores(b)
/ proj(a) / num(b) / proj(b), so each image's den->ln->1/den chain hides
behind the other image's matmuls.
"""

import numpy as np
import ml_dtypes
from contextlib import ExitStack

import concourse.bass as bass
import concourse.bacc as bacc
import concourse.tile as tile
import concourse.mybir as mybir
from concourse.bass_utils import run_bass_kernel_spmd

F32 = mybir.dt.float32
AF = mybir.ActivationFunctionType
OP = mybir.AluOpType
AX = mybir.AxisListType
DRM = mybir.MatmulPerfMode.DoubleRow

B, C, H, W = 16, 512, 32, 32
HW = H * W            # 1024
G = 32                # groupnorm groups
CPG = C // G          # 16 channels per group
EPS = 1e-5
NCORES = 8
BPC = B // NCORES     # 2 images per core
P = 128               # SBUF partitions
NCT = C // P          # 4 channel tiles
GPT = P // CPG        # 8 groups per channel tile
NSB = HW // P         # 8 spatial blocks of 128
FC = 512              # matmul moving-dim chunk (one PSUM bank of fp32)
NIC = HW // FC        # 2 chunks over the spatial free dim
WS = 32.0             # fp8 weight pre-scale (w*32 ~ N(0,1))
SM_SCALE = float(C) ** -0.5 / (WS * WS)   # exp scale; q,k each carry a 32x
NWARM = 25            # warm-up matmuls covering the DMA/stats front

DT = mybir.dt.bfloat16          # residual-adjacent dtype (x, NUM, wo, out)
DT_NP = ml_dtypes.bfloat16
F8 = mybir.dt.float8e4          # attention-internals dtype (DoubleRow matmuls)
F8_NP = ml_dtypes.float8_e4m3

_CACHE: dict = {}


def _pin_act_tables():
    """Narrow the ACT table map so exp/ln/square/identity/copy resolve only to
    natural_log_exp_and_others: the insert_act_table_loads pass then emits ONE
    table load instead of thrashing between exp_and_others and natural_log
    (~2.7us per swap). Set order (and so act_func_set_id) is preserved."""
    if _CACHE.get("tables_pinned"):
        return
    orig = bacc.get_activation_tables
    pinned = {AF.Exp, AF.Ln, AF.Square, AF.Identity, AF.Copy}

    def patched(arch):
        tabs = orig(arch)
        return {
            name: (fns if name == "natural_log_exp_and_others" else (fns - pinned))
            for name, fns in tabs.items()
        }

    bacc.get_activation_tables = patched
    _CACHE["tables_pinned"] = True


def _mm(nc, out, lhsT, rhs, start, stop):
    nc.tensor.matmul(out, lhsT, rhs, start=start, stop=stop)


def _mm8(nc, out, lhsT, rhs, start, stop):
    nc.tensor.matmul(out, lhsT, rhs, start=start, stop=stop, perf_mode=DRM)


def _emit(ctx, tc, io):
    nc = tc.nc

    consts = ctx.enter_context(tc.tile_pool(name="consts", bufs=1))
    pX16 = ctx.enter_context(tc.tile_pool(name="pX16", bufs=2))
    pXB = ctx.enter_context(tc.tile_pool(name="pXB", bufs=2))
    pHN = ctx.enter_context(tc.tile_pool(name="pHN", bufs=2))
    pQ = ctx.enter_context(tc.tile_pool(name="pQ", bufs=2))
    pK = ctx.enter_context(tc.tile_pool(name="pK", bufs=2))
    pVT = ctx.enter_context(tc.tile_pool(name="pVT", bufs=2))
    pPT = ctx.enter_context(tc.tile_pool(name="pPT", bufs=2))
    pNUM = ctx.enter_context(tc.tile_pool(name="pNUM", bufs=2))
    pOUT = ctx.enter_context(tc.tile_pool(name="pOUT", bufs=2))
    pS = ctx.enter_context(tc.tile_pool(name="pS", bufs=2))
    # 2-bank matmul tiles: [P, NIC, FC] fp32, 3 in flight + one aux ring
    pmm = ctx.enter_context(tc.tile_pool(name="pmm", bufs=3, space="PSUM"))
    paux = ctx.enter_context(tc.tile_pool(name="paux", bufs=1, space="PSUM"))

    # ---- both images' fp8 x copies go out FIRST as ONE packed descriptor
    # each (4KB per-partition rows: ~185 GB/s vs ~55 GB/s for 1KB rows).
    # Image a on sync (lands ~10us); image b on scalar, issued before any ACT
    # compute exists so the descriptor generation can't stall activations.
    def emit_load16(i, q):
        # two descriptors per image (2KB rows — still the fast DMA class):
        # stats on the first half start ~1.5us before the second half lands
        X16 = pX16.tile([P, NCT, HW], F8, name=f"X16_{i}", tag="X16")
        h = NCT // 2 * HW
        q.dma_start(X16[:, 0:NCT // 2, :], io["x16"][i, :, 0:h])
        q.dma_start(X16[:, NCT // 2:, :], io["x16"][i, :, h:])
        return X16

    X16_0 = emit_load16(0, nc.sync)

    def load_const(name, shape, dtype=F32, q=None):
        t = consts.tile(list(shape), dtype, name=f"c_{name}")
        (q or nc.sync).dma_start(t[:], io[name][:])
        return t

    # all (P, *) vectors packed into ONE DMA — each dma_start costs ~600ns of
    # issuing-engine descriptor time that would otherwise delay weight loads
    cvec = load_const("cvec", (P, 5 * NCT + GPT))
    bq_sb = cvec[:, 0 * NCT:1 * NCT]
    bk_sb = cvec[:, 1 * NCT:2 * NCT]
    bo_sb = cvec[:, 2 * NCT:3 * NCT]
    gs_sb = cvec[:, 3 * NCT:4 * NCT]
    gb_sb = cvec[:, 4 * NCT:5 * NCT]
    gsel = cvec[:, 5 * NCT:5 * NCT + GPT]
    gselT = load_const("gselT", (GPT, P))

    # ---- weights (loaded once, shared by both images), one packed DMA per
    # matrix — ALL on the sync queue: the SP engine has no compute, so its
    # descriptor issuance is free, while a dma_start on the scalar engine
    # blocks the ACT pipeline ~0.7us (and a full ring blocks it for the whole
    # transfer). Ordered by need time: wq (first Q matmul) -> wk -> wv.
    # q/k/v weights are fp8 (x32) with [P, ct, c_out] layout so a
    # [:, ct:ct+2, :] slice is a DoubleRow stationary operand; wo stays bf16.
    w_sb = {}
    for wname in ("wqt", "wkt", "wvt", "wot"):
        t = consts.tile([P, NCT, C], F8, name=f"{wname}_p")
        nc.sync.dma_start(t[:, :, :], io[wname][:])
        w_sb[wname] = t
    # image b's x16 lands ~17.5us — AFTER image a's norm chain is underway, so
    # the scheduler can't hoist b's stats squares ahead of a's Ln on the
    # in-order ACT queue (data-readiness drives its priorities)
    X16_1 = emit_load16(1, nc.sync)
    # image a's bf16 residual copy (with bo + wo@bv pre-folded on the host)
    # rides sync behind the weights (one 8KB-row descriptor, lands ~23us,
    # needed ~60us); registered to the image dict in emit_load below
    XB_0 = pXB.tile([P, NCT, HW], DT, name="XB0", tag="XB")
    nc.sync.dma_start(XB_0[:, :, :], io["x"][0])

    ones_col8 = consts.tile([P, 2, 16], F8, name="ones_col8")
    nc.vector.memset(ones_col8[:], WS)   # 32.0: cancels the 32x carried by VT
    ones_row = consts.tile([1, P], DT, name="ones_row")
    nc.vector.memset(ones_row[:], 1.0)
    zb = consts.tile([P, 1], F32, name="zb")
    nc.vector.memset(zb[:], 0.0)
    epsb = consts.tile([GPT, 1], F32, name="epsb")
    nc.vector.memset(epsb[:], EPS)
    # exp(-lnden + ln64) = 64/den: the 64x puts NUM in e4m3's normal range
    lb64 = consts.tile([P, 1], F32, name="lb64")
    nc.vector.memset(lb64[:], float(np.log(64.0)))
    # 1/(32*64): cancels NUM's 64x and wo's 32x at the final eviction
    inv2048 = consts.tile([P, 1], F32, name="inv2048")
    nc.vector.memset(inv2048[:], 1.0 / 2048.0)

    # ---- PE warm-up: a serial chain of matmuls spanning the DMA/stats front
    # keeps the HAM activity monitor busy so the clock gate opens to 2.4 GHz
    # (~3.4us in) and STAYS open until the first real matmul. Rotates through
    # the pmm ring so it costs no extra PSUM bank.
    warm8 = consts.tile([P, FC], F8, name="warm8")
    nc.vector.memset(warm8[:], 0.0)
    for w in range(NWARM):
        wp = pmm.tile([1, FC], F32, name=f"warm{w}", tag="mm")
        _mm(nc, wp[:], ones_col8[:, 0, 0:1], warm8[:], start=True, stop=True)

    # ---- per-image emission ----
    def new_img(i):
        return {"i": i}

    def emit_load(im):
        i = im["i"]
        im["X16"] = X16_0 if i == 0 else X16_1
        if i == 0:
            im["XB"] = XB_0
        else:
            # image b's residual copy rides the gpsimd software DGE (~90us of
            # slack before the proj(b) evictions need it)
            XB = pXB.tile([P, NCT, HW], DT, name=f"XB{i}", tag="XB")
            nc.gpsimd.dma_start(XB[:, :, :], io["x"][i])
            im["XB"] = XB

    def emit_stats(im):
        i = im["i"]
        X16 = im["X16"]
        stats = pS.tile([P, 2 * NCT], F32, name=f"stats{i}", tag="stats")
        scratch = pS.tile([P, HW], DT, name=f"scr{i}", tag="scratch")
        for ct in range(NCT):
            nc.vector.tensor_reduce(stats[:, ct:ct + 1], X16[:, ct, :], AX.X, OP.add)
            nc.scalar.activation(scratch[:], X16[:, ct, :], AF.Square, bias=zb[:],
                                 accum_out=stats[:, NCT + ct:NCT + ct + 1])
        im["stats"] = stats

    def emit_norm(im):
        # high_priority: the norm chain is ~12 tiny ops ping-ponging DVE<->ACT;
        # without it the scheduler interleaves the other image's 1.1us stats
        # passes between every step (+6us of pure latency on the critical path)
        i = im["i"]
        X16, stats = im["X16"], im["stats"]
        with nc.named_scope(f"norm{i}"), tc.high_priority():
            gst = paux.tile([GPT, 2 * NCT], F32, name=f"gst{i}", tag="aux")
            _mm(nc, gst[:], gsel[:], stats[:], start=True, stop=True)
            gm = pS.tile([GPT, 2 * NCT], F32, name=f"gm{i}", tag="gm")
            nc.vector.tensor_scalar_mul(gm[:], gst[:], 1.0 / (CPG * HW))
            sq = pS.tile([GPT, NCT], F32, name=f"sq{i}", tag="sq")
            nc.vector.tensor_mul(sq[:], gm[:, 0:NCT], gm[:, 0:NCT])
            var = pS.tile([GPT, NCT], F32, name=f"var{i}", tag="var")
            nc.vector.tensor_sub(var[:], gm[:, NCT:], sq[:])
            # rstd = exp(-0.5*ln(var+eps)) — Ln/Exp live in one ACT table set,
            # unlike Sqrt (whose set swap costs ~2.7us each way)
            lnv = pS.tile([GPT, NCT], F32, name=f"lnv{i}", tag="lnv")
            nc.scalar.activation(lnv[:], var[:], AF.Ln, bias=epsb[:])
            gmr = pS.tile([GPT, 2 * NCT], F32, name=f"gmr{i}", tag="gmr")
            nc.vector.tensor_copy(gmr[:, 0:NCT], gm[:, 0:NCT])
            nc.scalar.activation(gmr[:, NCT:], lnv[:], AF.Exp, bias=zb[0:GPT, :],
                                 scale=-0.5)
            pmr = paux.tile([P, 2 * NCT], F32, name=f"pmr{i}", tag="aux")
            _mm(nc, pmr[:], gselT[:], gmr[:], start=True, stop=True)
            mr = pS.tile([P, 2 * NCT], F32, name=f"mr{i}", tag="mr")
            nc.vector.tensor_copy(mr[:], pmr[:])
            # a = rstd*scale (cols NCT..), b = gn_bias - mean*a (cols 0..NCT)
            ab = pS.tile([P, 2 * NCT], F32, name=f"ab{i}", tag="ab")
            tb = pS.tile([P, NCT], F32, name=f"tb{i}", tag="tb")
            for ct in range(NCT):
                a_col = ab[:, NCT + ct:NCT + ct + 1]
                nc.vector.tensor_mul(a_col, mr[:, NCT + ct:NCT + ct + 1], gs_sb[:, ct:ct + 1])
                nc.vector.tensor_mul(tb[:, ct:ct + 1], mr[:, ct:ct + 1], a_col)
                nc.vector.tensor_sub(ab[:, ct:ct + 1], gb_sb[:, ct:ct + 1], tb[:, ct:ct + 1])
            HN = pHN.tile([P, NCT, HW], F8, name=f"HN{i}", tag="HN")
            for ct in range(NCT):
                nc.vector.tensor_scalar(HN[:, ct, :], X16[:, ct, :],
                                        ab[:, NCT + ct:NCT + ct + 1], ab[:, ct:ct + 1],
                                        OP.mult, OP.add)
            im["HN"] = HN

    def emit_qkv(im):
        i = im["i"]
        HN = im["HN"]
        with nc.named_scope(f"qkv{i}"):
            Q = pQ.tile([P, NCT, HW], F8, name=f"Q{i}", tag="Q")
            K = pK.tile([P, NCT, HW], F8, name=f"K{i}", tag="K")
            for wname, bias_sb, OT, on_act in (("wqt", bq_sb, Q, True),
                                               ("wkt", bk_sb, K, False)):
                for ob in range(NCT):
                    ps = pmm.tile([P, NIC, FC], F32, name=f"{wname}ps{i}_{ob}", tag="mm")
                    for ct in range(0, NCT, 2):
                        lhs = w_sb[wname][:, ct:ct + 2, ob * P:(ob + 1) * P]
                        for ic in range(NIC):
                            _mm8(nc, ps[:, ic, :], lhs, HN[:, ct:ct + 2, ic * FC:(ic + 1) * FC],
                                 start=(ct == 0), stop=(ct == NCT - 2))
                    # one [128,1024] eviction per ob; Q on ACT, K on DVE to
                    # balance the two engines' load
                    if on_act:
                        nc.scalar.add(OT[:, ob, :], ps[:], bias_sb[:, ob:ob + 1])
                    else:
                        nc.vector.tensor_scalar_add(OT[:, ob, :], ps[:],
                                                    bias_sb[:, ob:ob + 1])
            VT = pVT.tile([P, NSB, C], F8, name=f"VT{i}", tag="VT")
            for sb in range(0, NSB, 2):
                ps = pmm.tile([P, 2, C], F32, name=f"vtps{i}_{sb}", tag="mm")
                for k in range(2):
                    for ct in range(0, NCT, 2):
                        _mm8(nc, ps[:, k, :], HN[:, ct:ct + 2, (sb + k) * P:(sb + k + 1) * P],
                             w_sb["wvt"][:, ct:ct + 2, 0:C],
                             start=(ct == 0), stop=(ct == NCT - 2))
                nc.vector.tensor_copy(VT[:, sb:sb + 2, :], ps[:])
            im["Q"], im["K"], im["VT"] = Q, K, VT

    def emit_scores(im):
        i = im["i"]
        Q, K = im["Q"], im["K"]
        with nc.named_scope(f"scores{i}"):
            PT = pPT.tile([P, NSB, HW], F8, name=f"PT{i}", tag="PT")
            # den accumulates across jb pairs; its MMs are emitted inside the
            # jb loop so each lands right after the exp that feeds it
            den = paux.tile([1, NIC, FC], F32, name=f"den{i}", tag="aux")
            for jb in range(NSB):
                ps = pmm.tile([P, NIC, FC], F32, name=f"sps{i}_{jb}", tag="mm")
                for ct in range(0, NCT, 2):
                    lhs = K[:, ct:ct + 2, jb * P:(jb + 1) * P]
                    for ic in range(NIC):
                        _mm8(nc, ps[:, ic, :], lhs, Q[:, ct:ct + 2, ic * FC:(ic + 1) * FC],
                             start=(ct == 0), stop=(ct == NCT - 2))
                nc.scalar.activation(PT[:, jb, :], ps[:], AF.Exp, bias=zb[:],
                                     scale=SM_SCALE)
                if jb % 2 == 1:
                    for ic in range(NIC):
                        _mm8(nc, den[:, ic, :], ones_col8[:, 0:2, 0:1],
                             PT[:, jb - 1:jb + 1, ic * FC:(ic + 1) * FC],
                             start=(jb == 1), stop=(jb == NSB - 1))
            lnden = pS.tile([1, HW], DT, name=f"lnden{i}", tag="lnden")
            nc.scalar.activation(lnden[:], den[:], AF.Ln, bias=zb[0:1, :])
            im["PT"], im["lnden"] = PT, lnden

    def emit_attn_num(im):
        i = im["i"]
        VT, PT = im["VT"], im["PT"]
        with nc.named_scope(f"num{i}"):
            # num = vT.T @ P^T with the 1/den softmax normalization folded into
            # the PSUM eviction (commutes with the channel-wise wo projection);
            # 1/den arrives as exp(-lnden) with the broadcast done by a matmul
            # BETWEEN Ln and Exp so no engine touches 1 lane for long. The
            # Ln/rb/Exp chain hides behind the OTHER image's matmuls (qkv(b)
            # for image a, proj(a) for image b).
            recipb = pS.tile([P, HW], F32, name=f"recipb{i}", tag="recipb")
            rb = paux.tile([P, NIC, FC], F32, name=f"rb{i}", tag="aux")
            for ic in range(NIC):
                _mm(nc, rb[:, ic, :], ones_row[:],
                    im["lnden"][:, ic * FC:(ic + 1) * FC], start=True, stop=True)
            # 64/den: the 64x scales NUM into e4m3's normal range; cancelled
            # by the 1/2048 at the proj eviction
            nc.scalar.activation(recipb[:], rb[:], AF.Exp, bias=lb64[:], scale=-1.0)
            NUM = pNUM.tile([P, NCT, HW], F8, name=f"NUM{i}", tag="NUM")
            for cb in range(NCT):
                ps = pmm.tile([P, NIC, FC], F32, name=f"nps{i}_{cb}", tag="mm")
                for jt in range(0, NSB, 2):
                    lhs = VT[:, jt:jt + 2, cb * P:(cb + 1) * P]
                    for ic in range(NIC):
                        _mm8(nc, ps[:, ic, :], lhs, PT[:, jt:jt + 2, ic * FC:(ic + 1) * FC],
                             start=(jt == 0), stop=(jt == NSB - 2))
                nc.vector.tensor_mul(NUM[:, cb, :], ps[:], recipb[:])
            im["NUM"] = NUM

    def emit_attn_proj(im):
        i = im["i"]
        XB, NUM = im["XB"], im["NUM"]
        with nc.named_scope(f"proj{i}"):
            # proj (fp8 DoubleRow, carries 2048x) + residual straight from
            # PSUM: out = ps/2048 + xb, where xb already holds x + bo + wo@bv
            # (folded on the host — both commute past the attention average)
            OUTT = pOUT.tile([P, NCT, HW], DT, name=f"OUT{i}", tag="OUT")
            for ob in range(NCT):
                ps = pmm.tile([P, NIC, FC], F32, name=f"pps{i}_{ob}", tag="mm")
                for ct in range(0, NCT, 2):
                    lhs = w_sb["wot"][:, ct:ct + 2, ob * P:(ob + 1) * P]
                    for ic in range(NIC):
                        _mm8(nc, ps[:, ic, :], lhs, NUM[:, ct:ct + 2, ic * FC:(ic + 1) * FC],
                             start=(ct == 0), stop=(ct == NCT - 2))
                nc.vector.scalar_tensor_tensor(OUTT[:, ob, :], ps[:],
                                               inv2048[:, 0:1], XB[:, ob, :],
                                               OP.mult, OP.add)
                # image a stores on sync only (ACT is busy with scores(b) exps
                # then — a scalar dma_start would stall them); image b
                # alternates sync/scalar (ACT has no work left by then).
                # Full [128,1024] bf16 blocks keep 2KB per-partition rows.
                q = nc.sync if (i == 0 or ob % 2 == 0) else nc.scalar
                q.dma_start(io["out"][i, ob * P:(ob + 1) * P, :], OUTT[:, ob, :])

    ims = [new_img(i) for i in range(BPC)]
    a, b = ims
    emit_load(a)
    emit_stats(a)
    emit_load(b)
    emit_norm(a)
    emit_qkv(a)
    with tc.tile_wait_until(0.018):
        # schedule-time floor of ~18us: keeps b's 1.1us stats passes OUT of
        # the ACT/DVE streams while image a's latency-critical norm chain
        # ping-pongs between those engines (the scheduler's DMA cost model
        # is ~10x optimistic, so without this b's stats look ready at ~2us)
        emit_stats(b)
    emit_norm(b)
    emit_scores(a)
    emit_qkv(b)          # hides image a's den->ln->recip chain
    emit_attn_num(a)
    emit_scores(b)
    emit_attn_proj(a)    # hides image b's den->ln->recip chain
    emit_attn_num(b)
    emit_attn_proj(b)


def _build():
    if "nc" in _CACHE:
        return _CACHE["nc"]
    _pin_act_tables()
    nc = bacc.Bacc("TRN2", target_bir_lowering=False, debug=False, num_devices=NCORES)
    io = {}
    # x and x16 are host-packed to (P, NCT*HW) so each image loads as ONE
    # descriptor with >=4KB per-partition rows (~185 GB/s vs ~55 for 1KB rows)
    io["x"] = nc.dram_tensor("x", [BPC, P, NCT * HW], DT, kind="ExternalInput").ap()
    io["x16"] = nc.dram_tensor("x16", [BPC, P, NCT * HW], F8,
                               kind="ExternalInput").ap()
    for wname in ("wqt", "wkt", "wvt", "wot"):
        io[wname] = nc.dram_tensor(wname, [P, NCT, C], F8, kind="ExternalInput").ap()
    io["cvec"] = nc.dram_tensor("cvec", [P, 5 * NCT + GPT], F32,
                                kind="ExternalInput").ap()
    io["gselT"] = nc.dram_tensor("gselT", [GPT, P], F32, kind="ExternalInput").ap()
    io["out"] = nc.dram_tensor("out", [BPC, C, HW], DT, kind="ExternalOutput").ap()

    with tile.TileContext(nc) as tc:
        with ExitStack() as ctx:
            _emit(ctx, tc, io)
    nc.compile()
    _CACHE["nc"] = nc
    return nc


def _col_layout(v):
    # (C,) -> (P, NCT): column ct holds channels [ct*128, (ct+1)*128)
    return np.ascontiguousarray(np.asarray(v, np.float32).reshape(NCT, P).T)


def _run(inputs, trace=False, **run_kwargs):
    x = np.ascontiguousarray(np.asarray(inputs["x"], np.float32).reshape(B, C, HW))
    def _wpack(w, scale, npdt):
        # wT (c_in, c_out) -> (P, NCT, C): W[p, ct, j] = wT[ct*128+p, j] * scale
        wt = (np.asarray(w, np.float32).T * scale).astype(npdt)
        return np.ascontiguousarray(wt.reshape(NCT, P, C).transpose(1, 0, 2))

    wdt = {n: _wpack(inputs[s], WS, F8_NP)
           for n, s in (("wqt", "wq"), ("wkt", "wk"), ("wvt", "wv"), ("wot", "wo"))}
    pidx = np.arange(P)
    gsel = (pidx[:, None] // CPG == np.arange(GPT)[None, :]).astype(np.float32)
    # bq/bk carry the 32x weight scale so Q=32q, K=32k on-device; bv is folded
    # into bo_eff on-device (bo + wo@bv) so vT needs no bias at all
    cvec = np.concatenate([_col_layout(np.asarray(inputs["bq"]) * WS),
                           _col_layout(np.asarray(inputs["bk"]) * WS),
                           _col_layout(inputs["bo"]), _col_layout(inputs["gn_scale"]),
                           _col_layout(inputs["gn_bias"]), gsel], axis=1)
    common = {
        **wdt,
        "cvec": np.ascontiguousarray(cvec),
        "gselT": np.ascontiguousarray(gsel.T),
    }
    # bo + wo@bv commute past the attention average (sum_j P/den = 1), so the
    # whole channel-wise output shift folds into the residual copy on the host
    bo_eff = (np.asarray(inputs["bo"], np.float32)
              + np.asarray(inputs["wo"], np.float32)
              @ np.asarray(inputs["bv"], np.float32))
    # pack (B, C, HW) -> (B, P, NCT*HW): row p holds channels p, 128+p, ...
    xbo = x + bo_eff[None, :, None]
    xp = xbo.reshape(B, NCT, P, HW).transpose(0, 2, 1, 3).reshape(B, P, NCT * HW)
    xb = xp.astype(DT_NP)
    x16p = x.reshape(B, NCT, P, HW).transpose(0, 2, 1, 3).reshape(B, P, NCT * HW)
    x16 = x16p.astype(F8_NP)
    in_maps = [{"x": np.ascontiguousarray(xb[m * BPC:(m + 1) * BPC]),
                "x16": np.ascontiguousarray(x16[m * BPC:(m + 1) * BPC]), **common}
               for m in range(NCORES)]
    nc = _build()
    res = run_bass_kernel_spmd(nc, in_maps, core_ids=list(range(NCORES)),
                               trace=trace, **run_kwargs)
    out = np.concatenate([r["out"] for r in res.results], axis=0)
    return out.reshape(B, C, H, W).astype(np.float32), res


def kernel(**inputs):
    out, _ = _run(inputs)
    return out
